# revision 24
# baseline (speedup 1.0000x reference)
"""Trainium2 Bass kernel for nn_AdaptiveBlock (adaptive attention captioning block).

Data-parallel over batch across 8 NeuronCores (8 batches/core, weights
replicated). All GEMM operands in bf16 (host-cast); activations are supplied
pre-transposed [feature, rows] from the host so every matmul contracts over
the partition dim with both operands in natural DMA layout.

Loops are engine-homogeneous to avoid in-order head-of-line blocking (PE
transposes never sit between matmul groups and DVE chains):
  B1a: pg/pv matmuls + pv broadcast (PE + GpSimd)
  B1b: z cube + softmax(49) + alpha          (DVE + ACT only)
  A:   gates + sentinel (transposed)          (loads stream during B1)
  TR:  alpha^T PE transposes
  B2:  sentinel logit z_s + beta; batched beta broadcast via K=1 matmul
  C:   c_t + blend -> (c_hat + hiddens)^T
  D:   scores = act @ Wm (+ bm)
"""

import sys

for _p in ("/opt/trn_rl_repo",):
    if _p not in sys.path:
        sys.path.insert(0, _p)

import numpy as np
import ml_dtypes

BF16 = ml_dtypes.bfloat16

B, T, H, A, V1 = 64, 128, 1024, 49, 9488
N_CORES = 8
BL = B // N_CORES          # 8 batches per core
ROWS = BL * T              # 1024 rows (b-major, t-minor)
KC = H // 128              # 8 contraction chunks
RT = ROWS // 128           # 8 row tiles (tile rt == batch rt)
NV = 512                   # v-chunk width
NVC = (V1 + NV - 1) // NV  # 19 v-chunks (last = 272)


def build_nc(with_bias=False):
    import concourse.bacc as bacc
    import concourse.mybir as mybir
    import concourse.tile as tile

    dt = mybir.dt
    f32, bf = dt.float32, dt.bfloat16
    AF = mybir.ActivationFunctionType
    OP = mybir.AluOpType
    AX = mybir.AxisListType

    nc = bacc.Bacc("TRN2", target_bir_lowering=False, debug=False,
                   num_devices=N_CORES)

    # ---- DRAM parameters (per-core shard shapes, host-prepped layouts) ----
    xT_d = nc.dram_tensor("xT", [H, ROWS], bf, kind="ExternalInput")
    hT_d = nc.dram_tensor("hT", [H, ROWS], bf, kind="ExternalInput")
    cT_d = nc.dram_tensor("cT", [H, ROWS], bf, kind="ExternalInput")
    atT_d = nc.dram_tensor("atT", [H, BL * 64], bf, kind="ExternalInput")
    an_d = nc.dram_tensor("an", [BL * 64, H], bf, kind="ExternalInput")
    wxp = nc.dram_tensor("wxp", [KC, 128, KC, 128], bf, kind="ExternalInput")
    whp = nc.dram_tensor("whp", [KC, 128, KC, 128], bf, kind="ExternalInput")
    wv_d = nc.dram_tensor("wv", [H, A], bf, kind="ExternalInput")
    wg_d = nc.dram_tensor("wg", [H, A], bf, kind="ExternalInput")
    ws_d = nc.dram_tensor("ws", [H, A], bf, kind="ExternalInput")
    whr = nc.dram_tensor("whr", [128, A], f32, kind="ExternalInput")
    whc_d = nc.dram_tensor("whc", [1, A * A], f32, kind="ExternalInput")
    eye = nc.dram_tensor("eye", [128, 128], f32, kind="ExternalInput")
    wmp = nc.dram_tensor("wmp", [NVC, 128, KC, NV], bf, kind="ExternalInput")
    if with_bias:
        bmr_d = nc.dram_tensor("bmr", [NVC, 128, NV], f32, kind="ExternalInput")

    sc_o = nc.dram_tensor("scores", [ROWS, V1], f32, kind="ExternalOutput")
    al_o = nc.dram_tensor("alpha", [BL, T, A], f32, kind="ExternalOutput")
    be_o = nc.dram_tensor("beta", [BL, T, 1], f32, kind="ExternalOutput")

    def re3(ap):  # [H, N] dram -> [128, KC, N] tile order
        return ap.rearrange("(kc p) n -> p kc n", p=128)

    with tile.TileContext(nc) as tc:
        with (
            tc.tile_pool(name="const", bufs=1) as cpool,
            tc.tile_pool(name="persist", bufs=1) as pp,
            tc.tile_pool(name="psum_big", bufs=4, space="PSUM") as psb,
            tc.tile_pool(name="psum_small", bufs=4, space="PSUM") as pss,
        ):
            eye_sb = cpool.tile([128, 128], f32, tag="eye")
            nc.sync.dma_start(eye_sb[:, :], eye[:, :])
            whr_sb = cpool.tile([128, A], f32, tag="whr")
            nc.sync.dma_start(whr_sb[:, :], whr[:, :])
            ones_sb = cpool.tile([1, 128], f32, tag="ones")
            nc.vector.memset(ones_sb[:, :], 1.0)

            hT = pp.tile([128, KC, ROWS], bf, tag="hT")        # hiddens^T
            sT = pp.tile([128, KC, ROWS], bf, tag="sT")        # sentinel^T
            alT = pp.tile([128, BL, 128], bf, tag="alT")       # alpha^T (padded)
            brep = pp.tile([128, BL, 128], bf, tag="brep")    # beta replicated
            pg_sb = pp.tile([128, BL, A], f32, tag="pg")
            zs_sb = pp.tile([128, BL], f32, tag="zs")
            s49_keep = pp.tile([128, BL], f32, tag="s49k")
            bef_all = pp.tile([128, BL], f32, tag="befall")

            with tc.tile_pool(name="phB1l", bufs=1) as pb1l, \
                 tc.tile_pool(name="phAres", bufs=1) as pa:
                # scalar queue: B1-critical loads, then cT; compute-dependent
                # DMAs come only after all of these in program order.
                atT = pb1l.tile([128, KC, BL * 64], bf, tag="atT")
                whc_sb = pb1l.tile([128, A * A], f32, tag="whc")
                nc.scalar.dma_start(
                    whc_sb[:, :],
                    whc_d[0, :].unsqueeze(0).to_broadcast((128, A * A)))
                wv_sb = pb1l.tile([128, KC, A], bf, tag="wv")
                wg_sb = pb1l.tile([128, KC, A], bf, tag="wg")
                nc.scalar.dma_start(hT[:, :, :], re3(hT_d[:]))
                nc.scalar.dma_start(atT[:, :, :], re3(atT_d[:]))
                nc.scalar.dma_start(wv_sb[:, :, :], re3(wv_d[:]))
                nc.scalar.dma_start(wg_sb[:, :, :], re3(wg_d[:]))

                xT = pa.tile([128, KC, ROWS], bf, tag="xT")
                hpT = pa.tile([128, KC, ROWS], bf, tag="hpT")
                hpT3 = [hpT[:, kc, :].rearrange("p (b t) -> p b t", t=T)
                        for kc in range(KC)]
                hT3 = [hT[:, kc, :].rearrange("p (b t) -> p b t", t=T)
                       for kc in range(KC)]
                cT = pa.tile([128, KC, ROWS], bf, tag="cT")
                ws_sb = pa.tile([128, KC, A], bf, tag="ws")
                nc.sync.dma_start(xT[:, :, :], re3(xT_d[:]))
                nc.scalar.dma_start(ws_sb[:, :, :], re3(ws_d[:]))
                nc.scalar.dma_start(cT[:, :, :], re3(cT_d[:]))

                # ---------- B1a: pg/pv matmuls + pv broadcast ----------
                # All 16 small accumulation chains share 4 PSUM bank tiles
                # (4 chains per [128,512] bank) so every chain can be in
                # flight at once -- avoids slot-starvation pacing the PE.
                with tc.tile_pool(name="phB1w", bufs=3) as pb, \
                     tc.tile_pool(name="phB1big", bufs=2) as pbg:
                    pvcs = []
                    for g in range(4):
                        ps4 = psb.tile([128, 512], f32, tag="mmbig")
                        for j in range(2):
                            b = g * 2 + j
                            bs = slice(b * 128, (b + 1) * 128)
                            go = j * 256
                            for kc in range(KC):
                                nc.tensor.matmul(
                                    ps4[:, go:go + A], hT[:, kc, bs],
                                    wg_sb[:, kc, :],
                                    start=(kc == 0), stop=(kc == KC - 1))
                            for kc in range(KC):
                                nc.tensor.matmul(
                                    ps4[:49, go + 128:go + 128 + A],
                                    atT[:, kc, b * 64:b * 64 + 49],
                                    wv_sb[:, kc, :],
                                    start=(kc == 0), stop=(kc == KC - 1))
                        for j in range(2):
                            b = g * 2 + j
                            go = j * 256
                            nc.vector.tensor_copy(pg_sb[:, b, :],
                                                  ps4[:, go:go + A])
                            pv_sb = pb.tile([49, A], f32, tag="pv")
                            nc.vector.tensor_copy(pv_sb[:, :],
                                                  ps4[:49, go + 128:go + 128 + A])
                            pv_flat = pb.tile([1, A * A], f32, tag="pvf")
                            nc.gpsimd.dma_start(pv_flat[:, :], pv_sb[:, :])
                            pvc = pbg.tile([128, A * A], f32, tag="pvc")
                            nc.gpsimd.partition_broadcast(pvc[:, :],
                                                          pv_flat[:, :])
                            pvcs.append(pvc)

                    # ---------- A + B1b interleaved ----------
                    # Per-iteration: one gates group (PE-heavy) then one cube
                    # chain (DVE/ACT-heavy) so no engine queue gets a long
                    # head-of-line block from the other stream.
                    def cube_chain(b):
                        uc = pbg.tile([128, A, A], f32, tag="uc")
                        pg_b = pg_sb[:, b, :].unsqueeze(1).to_broadcast((128, A, A))
                        cube = pvcs[b][:, :].rearrange("p (r a) -> p r a", a=A)
                        nc.vector.tensor_add(uc[:, :, :], cube, pg_b)
                        nc.scalar.activation(uc[:, :, :], uc[:, :, :], AF.Tanh)
                        uflat = uc[:, :, :].rearrange("p r a -> p (r a)")
                        if b % 2 == 0:
                            nc.gpsimd.tensor_mul(uflat, uflat, whc_sb[:, :])
                        else:
                            wh_b = whr_sb[:, :].unsqueeze(1).to_broadcast(
                                (128, A, A))
                            nc.vector.tensor_mul(uc[:, :, :], uc[:, :, :], wh_b)
                        z_sb = pb.tile([128, A], f32, tag="z")
                        nc.vector.tensor_reduce(z_sb[:, :], uc[:, :, :], axis=AX.X,
                                                op=OP.add)
                        ez = pb.tile([128, A], f32, tag="ez")
                        nc.scalar.activation(ez[:, :], z_sb[:, :], AF.Exp)
                        s49 = pb.tile([128, 1], f32, tag="s49")
                        nc.vector.tensor_reduce(s49[:, :], ez[:, :], axis=AX.X,
                                                op=OP.add)
                        r49 = pb.tile([128, 1], f32, tag="r49")
                        nc.vector.reciprocal(r49[:, :], s49[:, :])
                        alf = pb.tile([128, A], f32, tag="alf")
                        nc.vector.tensor_scalar_mul(alf[:, :], ez[:, :], r49[:, :])
                        nc.scalar.dma_start(al_o[b], alf[:, :])
                        nc.vector.tensor_copy(s49_keep[:, b:b + 1], s49[:, :])
                        # alpha^T via DMA transpose (keeps the PE stream clean)
                        alfb = pb.tile([128, 128], bf, tag="alfb")
                        nc.vector.memset(alfb[:, A:], 0.0)
                        nc.vector.tensor_copy(alfb[:, :A], alf[:, :])
                        nc.scalar.dma_start(alT[:, b, :], alfb[:, :],
                                            transpose=True)

                    def a_group(hc, paw, pas):
                        wxs = pas.tile([128, KC, 128], bf, tag="wxs")
                        nc.sync.dma_start(wxs[:, :, :], wxp[hc])
                        whs = pas.tile([128, KC, 128], bf, tag="whs")
                        nc.sync.dma_start(whs[:, :, :], whp[hc])
                        if hc == 0:
                            for kc in range(KC):
                                nc.vector.memset(hpT3[kc][:, :, 0:1], 0.0)
                                nc.sync.dma_start(hpT3[kc][:, :, 1:T],
                                                  hT3[kc][:, :, 0:T - 1])
                        for rc in range(2):
                            rs = slice(rc * 512, (rc + 1) * 512)
                            ps = psb.tile([128, 512], f32, tag="mmbig")
                            for kc in range(KC):
                                nc.tensor.matmul(
                                    ps[:, :], wxs[:, kc, :], xT[:, kc, rs],
                                    start=(kc == 0), stop=False)
                            for kc in range(KC):
                                nc.tensor.matmul(
                                    ps[:, :], whs[:, kc, :], hpT[:, kc, rs],
                                    start=False, stop=(kc == KC - 1))
                            gt = paw.tile([128, 512], f32, tag="gt")
                            nc.scalar.activation(gt[:, :], ps[:, :], AF.Sigmoid)
                            tcl = paw.tile([128, 512], f32, tag="tcl")
                            nc.scalar.activation(tcl[:, :], cT[:, hc, rs], AF.Tanh)
                            nc.vector.tensor_mul(sT[:, hc, rs], gt[:, :], tcl[:, :])

                    with tc.tile_pool(name="phAw", bufs=3) as paw, \
                         tc.tile_pool(name="phAs", bufs=3) as pas:
                        for i in range(KC):
                            a_group(i, paw, pas)
                            cube_chain(i)

                # ---------- B2: z_s, beta ----------
                with tc.tile_pool(name="phB2", bufs=4) as pb2:
                    us_all = pb2.tile([128, BL, A], f32, tag="usall")
                    for b in range(BL):
                        bs = slice(b * 128, (b + 1) * 128)
                        pssn = pss.tile([128, 128], f32, tag="mmsmall")
                        for kc in range(KC):
                            nc.tensor.matmul(pssn[:, :A], sT[:, kc, bs],
                                             ws_sb[:, kc, :],
                                             start=(kc == 0), stop=(kc == KC - 1))
                        nc.vector.tensor_add(us_all[:, b, :], pssn[:, :A],
                                             pg_sb[:, b, :])
                    for b in range(BL):
                        us = us_all[:, b, :]
                        nc.scalar.activation(us, us, AF.Tanh)
                        scr = pb2.tile([128, A], f32, tag="scr")
                        nc.vector.tensor_mul(scr[:, :], us, whr_sb[:, :])
                        nc.vector.tensor_reduce(zs_sb[:, b:b + 1], scr[:, :],
                                                axis=AX.X, op=OP.add)
                        ezs = pb2.tile([128, 1], f32, tag="ezs")
                        nc.scalar.activation(ezs[:, :], zs_sb[:, b:b + 1], AF.Exp)
                        s50 = pb2.tile([128, 1], f32, tag="s50")
                        nc.vector.tensor_add(s50[:, :], s49_keep[:, b:b + 1],
                                             ezs[:, :])
                        r50 = pb2.tile([128, 1], f32, tag="r50")
                        nc.vector.reciprocal(r50[:, :], s50[:, :])
                        nc.vector.tensor_mul(bef_all[:, b:b + 1], ezs[:, :],
                                             r50[:, :])
                        nc.scalar.dma_start(be_o[b], bef_all[:, b:b + 1])

                    # batched beta broadcast: transpose + K=1 matmul replicate
                    psq = pss.tile([128, 128], f32, tag="mmsmall")
                    nc.tensor.transpose(psq[:BL, :], bef_all[:, :], eye_sb[:, :])
                    bT_sb = pb2.tile([BL, 128], f32, tag="bT")
                    nc.vector.tensor_copy(bT_sb[:, :], psq[:BL, :])
                    bfl = pb2.tile([1, BL * 128], f32, tag="bfl")
                    nc.gpsimd.dma_start(bfl[:, :], bT_sb[:, :])
                    for half in range(2):
                        hs = slice(half * 512, (half + 1) * 512)
                        psr = psb.tile([128, 512], f32, tag="mmbig")
                        nc.tensor.matmul(psr[:, :], ones_sb[:, :], bfl[:, hs],
                                         start=True, stop=True)
                        nc.vector.tensor_copy(
                            brep[:, half * 4:(half + 1) * 4, :].rearrange(
                                "p b t -> p (b t)"), psr[:, :])

            # ---------- C: c_t + blend -> actT ----------
            with tc.tile_pool(name="pc2", bufs=1) as pp2, \
                 tc.tile_pool(name="phC", bufs=3) as pc:
                aT = pp2.tile([128, KC, ROWS], bf, tag="aT")
                an = pp2.tile([64, BL, H], bf, tag="an")
                nc.scalar.dma_start(an[:, :, :],
                                    an_d[:].rearrange("(b r) k -> r b k", r=64))
                for bg in range(2):
                    for hc in range(KC):
                        gs = slice(bg * 512, (bg + 1) * 512)
                        ps = psb.tile([128, 512], f32, tag="mmbig")
                        for j in range(4):
                            b = bg * 4 + j
                            nc.tensor.matmul(
                                ps[:, j * 128:(j + 1) * 128],
                                an[:49, b, hc * 128:(hc + 1) * 128],
                                alT[:49, b, :], start=True, stop=True)
                        # c_hat = c_t + beta*(s - c_t);  act = c_hat + hiddens
                        ctb = pc.tile([128, 512], bf, tag="ctb")
                        nc.vector.tensor_copy(ctb[:, :], ps[:, :])
                        tmp = pc.tile([128, 512], bf, tag="tmp")
                        nc.vector.tensor_sub(tmp[:, :], sT[:, hc, gs], ctb[:, :])
                        br = brep[:, bg * 4:(bg + 1) * 4, :].rearrange(
                            "p b t -> p (b t)")
                        nc.vector.tensor_mul(tmp[:, :], tmp[:, :], br)
                        nc.vector.tensor_add(tmp[:, :], tmp[:, :], ctb[:, :])
                        nc.vector.tensor_add(aT[:, hc, gs], tmp[:, :], hT[:, hc, gs])

                # ---------- D: scores = act @ Wm (+ bm) ----------
                with tc.tile_pool(name="phD", bufs=3) as pd, \
                     tc.tile_pool(name="phDo", bufs=2) as pdo:
                    for vc in range(NVC):
                        nv = min(NV, V1 - vc * NV)
                        wm_t = pd.tile([128, KC, NV], bf, tag="wmt")
                        nc.sync.dma_start(wm_t[:, :, :], wmp[vc])
                        if with_bias:
                            bmr_t = pd.tile([128, NV], f32, tag="bmrt")
                            nc.sync.dma_start(bmr_t[:, :nv], bmr_d[vc, :, :nv])
                        for rg in range(2):
                            ot = pdo.tile([128, 4, NV], f32, tag="ot")
                            for j in range(4):
                                rt = rg * 4 + j
                                ps = psb.tile([128, 512], f32, tag="mmbig")
                                for kc in range(KC):
                                    nc.tensor.matmul(
                                        ps[:, :nv],
                                        aT[:, kc, rt * 128:(rt + 1) * 128],
                                        wm_t[:, kc, :nv],
                                        start=(kc == 0), stop=(kc == KC - 1))
                                if with_bias:
                                    nc.vector.tensor_add(ot[:, j, :nv], ps[:, :nv],
                                                         bmr_t[:, :nv])
                                else:
                                    nc.vector.tensor_copy(ot[:, j, :nv], ps[:, :nv])
                            dst = sc_o[rg * 512:(rg + 1) * 512,
                                       vc * NV:vc * NV + nv].rearrange(
                                "(rb p) c -> p rb c", p=128)
                            eng = nc.scalar if (vc + rg) % 2 else nc.sync
                            eng.dma_start(dst, ot[:, :, :nv])

    nc.compile()
    return nc


def prep_core_inputs(x, hiddens, cells, att_feats, Wx, Wh, Wv, Wg, Ws, wh, Wm, bm):
    """Host-side prep: bf16 casts, transposes, pads, weight retiling."""
    def bt(a):  # [B,T,H] f32 -> bf16 [B, H, T]
        return np.ascontiguousarray(
            a.astype(BF16).reshape(B, T, H).transpose(0, 2, 1))

    xTt = bt(x)
    hTt = bt(hiddens)
    cTt = bt(cells)

    ap = np.zeros((B, 64, H), BF16)
    ap[:, :A, :] = att_feats.astype(BF16)
    atTt = np.ascontiguousarray(ap.transpose(0, 2, 1))  # [B, H, 64]

    def wtile(W):  # [H, H] -> [hc, p, kc, c]
        return np.ascontiguousarray(
            W.astype(BF16).reshape(KC, 128, KC, 128).transpose(2, 1, 0, 3))

    wm_pad = np.zeros((H, NVC * NV), BF16)
    wm_pad[:, :V1] = Wm.astype(BF16)
    wmp = np.ascontiguousarray(
        wm_pad.reshape(KC, 128, NVC, NV).transpose(2, 1, 0, 3))

    with_bias = bool(np.any(bm))
    shared = {
        "wxp": wtile(Wx), "whp": wtile(Wh),
        "wv": Wv.astype(BF16), "wg": Wg.astype(BF16), "ws": Ws.astype(BF16),
        "whr": np.ascontiguousarray(
            np.broadcast_to(wh.astype(np.float32)[None, :], (128, A))),
        "whc": np.ascontiguousarray(
            np.tile(wh.astype(np.float32), A)[None, :]),
        "eye": np.eye(128, dtype=np.float32),
        "wmp": wmp,
    }
    if with_bias:
        bm_pad = np.zeros((NVC * NV,), np.float32)
        bm_pad[:V1] = bm.astype(np.float32)
        shared["bmr"] = np.ascontiguousarray(
            np.broadcast_to(bm_pad.reshape(NVC, 1, NV), (NVC, 128, NV)))

    def tocore(a, i):  # [B, H, N] -> [H, BL*N] for core i
        return np.ascontiguousarray(
            a[i * BL:(i + 1) * BL].transpose(1, 0, 2).reshape(H, -1))

    def core_map(i):
        m = dict(shared)
        m["xT"] = tocore(xTt, i)
        m["hT"] = tocore(hTt, i)
        m["cT"] = tocore(cTt, i)
        m["atT"] = tocore(atTt, i)
        m["an"] = ap[i * BL:(i + 1) * BL].reshape(BL * 64, H)
        return m

    return core_map, with_bias


def kernel(x, hiddens, cells, att_feats, Wx, Wh, Wv, Wg, Ws, wh, Wm, bm):
    from concourse.bass_utils import run_bass_kernel_spmd

    core_map, with_bias = prep_core_inputs(x, hiddens, cells, att_feats, Wx, Wh,
                                           Wv, Wg, Ws, wh, Wm, bm)
    nc = build_nc(with_bias=with_bias)
    in_maps = [core_map(i) for i in range(N_CORES)]
    res = run_bass_kernel_spmd(nc, in_maps, core_ids=list(range(N_CORES)))

    scores = np.concatenate(
        [res.results[i]["scores"].reshape(BL, T, V1) for i in range(N_CORES)], axis=0)
    alpha = np.concatenate(
        [res.results[i]["alpha"] for i in range(N_CORES)], axis=0)
    beta = np.concatenate(
        [res.results[i]["beta"] for i in range(N_CORES)], axis=0)
    return scores, alpha, beta


# revision 25
# speedup vs baseline: 1.1480x; 1.1480x over previous
"""Trainium2 Bass kernel for nn_AdaptiveBlock (adaptive attention captioning block).

Data-parallel over batch across 8 NeuronCores (8 batches/core, weights
replicated). All GEMM operands in bf16 (host-cast); activations are supplied
pre-transposed [feature, rows] from the host so every matmul contracts over
the partition dim with both operands in natural DMA layout.

Loops are engine-homogeneous to avoid in-order head-of-line blocking (PE
transposes never sit between matmul groups and DVE chains):
  B1a: pg/pv matmuls + pv broadcast (PE + GpSimd)
  B1b: z cube + softmax(49) + alpha          (DVE + ACT only)
  A:   gates + sentinel (transposed)          (loads stream during B1)
  TR:  alpha^T PE transposes
  B2:  sentinel logit z_s + beta; batched beta broadcast via K=1 matmul
  C:   c_t + blend -> (c_hat + hiddens)^T
  D:   scores = act @ Wm (+ bm)
"""

import sys

for _p in ("/opt/trn_rl_repo",):
    if _p not in sys.path:
        sys.path.insert(0, _p)

import numpy as np
import ml_dtypes

BF16 = ml_dtypes.bfloat16

B, T, H, A, V1 = 64, 128, 1024, 49, 9488
N_CORES = 8
BL = B // N_CORES          # 8 batches per core
ROWS = BL * T              # 1024 rows (b-major, t-minor)
KC = H // 128              # 8 contraction chunks
RT = ROWS // 128           # 8 row tiles (tile rt == batch rt)
NV = 512                   # v-chunk width
NVC = (V1 + NV - 1) // NV  # 19 v-chunks (last = 272)


def build_nc(with_bias=False):
    import concourse.bacc as bacc
    import concourse.mybir as mybir
    import concourse.tile as tile

    dt = mybir.dt
    f32, bf = dt.float32, dt.bfloat16
    AF = mybir.ActivationFunctionType
    OP = mybir.AluOpType
    AX = mybir.AxisListType

    nc = bacc.Bacc("TRN2", target_bir_lowering=False, debug=False,
                   num_devices=N_CORES)

    # ---- DRAM parameters (per-core shard shapes, host-prepped layouts) ----
    xT_d = nc.dram_tensor("xT", [H, ROWS], bf, kind="ExternalInput")
    hT_d = nc.dram_tensor("hT", [H, ROWS], bf, kind="ExternalInput")
    cT_d = nc.dram_tensor("cT", [H, ROWS], bf, kind="ExternalInput")
    atT_d = nc.dram_tensor("atT", [H, BL * 64], bf, kind="ExternalInput")
    an_d = nc.dram_tensor("an", [BL * 64, H], bf, kind="ExternalInput")
    wxp = nc.dram_tensor("wxp", [KC, 128, KC, 128], bf, kind="ExternalInput")
    whp = nc.dram_tensor("whp", [KC, 128, KC, 128], bf, kind="ExternalInput")
    wv_d = nc.dram_tensor("wv", [H, A], bf, kind="ExternalInput")
    wg_d = nc.dram_tensor("wg", [H, A], bf, kind="ExternalInput")
    ws_d = nc.dram_tensor("ws", [H, A], bf, kind="ExternalInput")
    whr = nc.dram_tensor("whr", [128, A], f32, kind="ExternalInput")
    eye = nc.dram_tensor("eye", [128, 128], f32, kind="ExternalInput")
    wmp = nc.dram_tensor("wmp", [NVC, 128, KC, NV], bf, kind="ExternalInput")
    if with_bias:
        bmr_d = nc.dram_tensor("bmr", [NVC, 128, NV], f32, kind="ExternalInput")

    sc_o = nc.dram_tensor("scores", [ROWS, V1], f32, kind="ExternalOutput")
    al_o = nc.dram_tensor("alpha", [BL, T, A], f32, kind="ExternalOutput")
    be_o = nc.dram_tensor("beta", [BL, T, 1], f32, kind="ExternalOutput")

    def re3(ap):  # [H, N] dram -> [128, KC, N] tile order
        return ap.rearrange("(kc p) n -> p kc n", p=128)

    with tile.TileContext(nc) as tc:
        with (
            tc.tile_pool(name="const", bufs=1) as cpool,
            tc.tile_pool(name="persist", bufs=1) as pp,
            tc.tile_pool(name="psum_big", bufs=4, space="PSUM") as psb,
            tc.tile_pool(name="psum_small", bufs=4, space="PSUM") as pss,
        ):
            eye_sb = cpool.tile([128, 128], f32, tag="eye")
            nc.sync.dma_start(eye_sb[:, :], eye[:, :])
            whr_sb = cpool.tile([128, A], f32, tag="whr")
            nc.sync.dma_start(whr_sb[:, :], whr[:, :])
            ones_sb = cpool.tile([1, 128], f32, tag="ones")
            nc.vector.memset(ones_sb[:, :], 1.0)

            hT = pp.tile([128, KC, ROWS], bf, tag="hT")        # hiddens^T
            sT = pp.tile([128, KC, ROWS], bf, tag="sT")        # sentinel^T
            alT = pp.tile([128, BL, 128], bf, tag="alT")       # alpha^T (padded)
            brep = pp.tile([128, BL, 128], bf, tag="brep")    # beta replicated
            pg_sb = pp.tile([128, BL, A], f32, tag="pg")
            zs_sb = pp.tile([128, BL], f32, tag="zs")
            s49_keep = pp.tile([128, BL], f32, tag="s49k")
            bef_all = pp.tile([128, BL], f32, tag="befall")

            with tc.tile_pool(name="phB1l", bufs=1) as pb1l, \
                 tc.tile_pool(name="phAres", bufs=1) as pa:
                # scalar queue: B1-critical loads, then cT; compute-dependent
                # DMAs come only after all of these in program order.
                atT = pb1l.tile([128, KC, BL * 64], bf, tag="atT")

                wv_sb = pb1l.tile([128, KC, A], bf, tag="wv")
                wg_sb = pb1l.tile([128, KC, A], bf, tag="wg")
                nc.scalar.dma_start(hT[:, :, :], re3(hT_d[:]))
                nc.scalar.dma_start(atT[:, :, :], re3(atT_d[:]))
                nc.scalar.dma_start(wv_sb[:, :, :], re3(wv_d[:]))
                nc.scalar.dma_start(wg_sb[:, :, :], re3(wg_d[:]))

                xT = pa.tile([128, KC, ROWS], bf, tag="xT")
                hpT = pa.tile([128, KC, ROWS], bf, tag="hpT")
                hpT3 = [hpT[:, kc, :].rearrange("p (b t) -> p b t", t=T)
                        for kc in range(KC)]
                hT3 = [hT[:, kc, :].rearrange("p (b t) -> p b t", t=T)
                       for kc in range(KC)]
                cT = pa.tile([128, KC, ROWS], bf, tag="cT")
                ws_sb = pa.tile([128, KC, A], bf, tag="ws")
                nc.sync.dma_start(xT[:, :, :], re3(xT_d[:]))
                nc.scalar.dma_start(ws_sb[:, :, :], re3(ws_d[:]))
                nc.scalar.dma_start(cT[:, :, :], re3(cT_d[:]))

                # ---------- B1a: pg/pv matmuls + pv broadcast ----------
                # All 16 small accumulation chains share 4 PSUM bank tiles
                # (4 chains per [128,512] bank) so every chain can be in
                # flight at once -- avoids slot-starvation pacing the PE.
                with tc.tile_pool(name="phB1w", bufs=3) as pb, \
                     tc.tile_pool(name="phB1big", bufs=2) as pbg:
                    pvcs = []
                    for g in range(4):
                        ps4 = psb.tile([128, 512], f32, tag="mmbig")
                        for j in range(2):
                            b = g * 2 + j
                            bs = slice(b * 128, (b + 1) * 128)
                            go = j * 256
                            for kc in range(KC):
                                nc.tensor.matmul(
                                    ps4[:, go:go + A], hT[:, kc, bs],
                                    wg_sb[:, kc, :],
                                    start=(kc == 0), stop=(kc == KC - 1))
                            for kc in range(KC):
                                nc.tensor.matmul(
                                    ps4[:49, go + 128:go + 128 + A],
                                    atT[:, kc, b * 64:b * 64 + 49],
                                    wv_sb[:, kc, :],
                                    start=(kc == 0), stop=(kc == KC - 1))
                        for j in range(2):
                            b = g * 2 + j
                            go = j * 256
                            nc.vector.tensor_copy(pg_sb[:, b, :],
                                                  ps4[:, go:go + A])
                            pv_sb = pb.tile([49, A], f32, tag="pv")
                            nc.vector.tensor_copy(pv_sb[:, :],
                                                  ps4[:49, go + 128:go + 128 + A])
                            pv_flat = pb.tile([1, A * A], f32, tag="pvf")
                            nc.gpsimd.dma_start(pv_flat[:, :], pv_sb[:, :])
                            pvc = pbg.tile([128, A * A], f32, tag="pvc")
                            nc.gpsimd.partition_broadcast(pvc[:, :],
                                                          pv_flat[:, :])
                            pvcs.append(pvc)

                    # ---------- A + B1b interleaved ----------
                    # Per-iteration: one gates group (PE-heavy) then one cube
                    # chain (DVE/ACT-heavy) so no engine queue gets a long
                    # head-of-line block from the other stream.
                    def cube_chain(b):
                        uc = pbg.tile([128, A, A], f32, tag="uc")
                        pg_b = pg_sb[:, b, :].unsqueeze(1).to_broadcast((128, A, A))
                        cube = pvcs[b][:, :].rearrange("p (r a) -> p r a", a=A)
                        nc.vector.tensor_add(uc[:, :, :], cube, pg_b)
                        nc.scalar.activation(uc[:, :, :], uc[:, :, :], AF.Tanh)
                        wh_b = whr_sb[:, :].unsqueeze(1).to_broadcast((128, A, A))
                        nc.vector.tensor_mul(uc[:, :, :], uc[:, :, :], wh_b)
                        z_sb = pb.tile([128, A], f32, tag="z")
                        nc.vector.tensor_reduce(z_sb[:, :], uc[:, :, :], axis=AX.X,
                                                op=OP.add)
                        ez = pb.tile([128, A], f32, tag="ez")
                        nc.scalar.activation(ez[:, :], z_sb[:, :], AF.Exp)
                        s49 = pb.tile([128, 1], f32, tag="s49")
                        nc.vector.tensor_reduce(s49[:, :], ez[:, :], axis=AX.X,
                                                op=OP.add)
                        r49 = pb.tile([128, 1], f32, tag="r49")
                        nc.vector.reciprocal(r49[:, :], s49[:, :])
                        alf = pb.tile([128, A], f32, tag="alf")
                        nc.vector.tensor_scalar_mul(alf[:, :], ez[:, :], r49[:, :])
                        nc.scalar.dma_start(al_o[b], alf[:, :])
                        nc.vector.tensor_copy(s49_keep[:, b:b + 1], s49[:, :])
                        # alpha^T via DMA transpose (keeps the PE stream clean)
                        alfb = pb.tile([128, 128], bf, tag="alfb")
                        nc.vector.memset(alfb[:, A:], 0.0)
                        nc.vector.tensor_copy(alfb[:, :A], alf[:, :])
                        nc.scalar.dma_start(alT[:, b, :], alfb[:, :],
                                            transpose=True)

                    def a_group(hc, paw, pas):
                        wxs = pas.tile([128, KC, 128], bf, tag="wxs")
                        nc.sync.dma_start(wxs[:, :, :], wxp[hc])
                        whs = pas.tile([128, KC, 128], bf, tag="whs")
                        nc.sync.dma_start(whs[:, :, :], whp[hc])
                        if hc == 0:
                            for kc in range(KC):
                                nc.vector.memset(hpT3[kc][:, :, 0:1], 0.0)
                                nc.sync.dma_start(hpT3[kc][:, :, 1:T],
                                                  hT3[kc][:, :, 0:T - 1])
                        for rc in range(2):
                            rs = slice(rc * 512, (rc + 1) * 512)
                            ps = psb.tile([128, 512], f32, tag="mmbig")
                            for kc in range(KC):
                                nc.tensor.matmul(
                                    ps[:, :], wxs[:, kc, :], xT[:, kc, rs],
                                    start=(kc == 0), stop=False)
                            for kc in range(KC):
                                nc.tensor.matmul(
                                    ps[:, :], whs[:, kc, :], hpT[:, kc, rs],
                                    start=False, stop=(kc == KC - 1))
                            gt = paw.tile([128, 512], f32, tag="gt")
                            nc.scalar.activation(gt[:, :], ps[:, :], AF.Sigmoid)
                            tcl = paw.tile([128, 512], f32, tag="tcl")
                            nc.scalar.activation(tcl[:, :], cT[:, hc, rs], AF.Tanh)
                            nc.vector.tensor_mul(sT[:, hc, rs], gt[:, :], tcl[:, :])

                    with tc.tile_pool(name="phAw", bufs=3) as paw, \
                         tc.tile_pool(name="phAs", bufs=4) as pas:
                        for i in range(KC):
                            a_group(i, paw, pas)
                            cube_chain(i)

                # ---------- B2: z_s, beta ----------
                with tc.tile_pool(name="phB2", bufs=4) as pb2:
                    us_all = pb2.tile([128, BL, A], f32, tag="usall")
                    for b in range(BL):
                        bs = slice(b * 128, (b + 1) * 128)
                        pssn = pss.tile([128, 128], f32, tag="mmsmall")
                        for kc in range(KC):
                            nc.tensor.matmul(pssn[:, :A], sT[:, kc, bs],
                                             ws_sb[:, kc, :],
                                             start=(kc == 0), stop=(kc == KC - 1))
                        nc.vector.tensor_add(us_all[:, b, :], pssn[:, :A],
                                             pg_sb[:, b, :])
                    for b in range(BL):
                        us = us_all[:, b, :]
                        nc.scalar.activation(us, us, AF.Tanh)
                        scr = pb2.tile([128, A], f32, tag="scr")
                        nc.vector.tensor_mul(scr[:, :], us, whr_sb[:, :])
                        nc.vector.tensor_reduce(zs_sb[:, b:b + 1], scr[:, :],
                                                axis=AX.X, op=OP.add)
                        ezs = pb2.tile([128, 1], f32, tag="ezs")
                        nc.scalar.activation(ezs[:, :], zs_sb[:, b:b + 1], AF.Exp)
                        s50 = pb2.tile([128, 1], f32, tag="s50")
                        nc.vector.tensor_add(s50[:, :], s49_keep[:, b:b + 1],
                                             ezs[:, :])
                        r50 = pb2.tile([128, 1], f32, tag="r50")
                        nc.vector.reciprocal(r50[:, :], s50[:, :])
                        nc.vector.tensor_mul(bef_all[:, b:b + 1], ezs[:, :],
                                             r50[:, :])
                        nc.scalar.dma_start(be_o[b], bef_all[:, b:b + 1])

                    # batched beta broadcast: transpose + K=1 matmul replicate
                    psq = pss.tile([128, 128], f32, tag="mmsmall")
                    nc.tensor.transpose(psq[:BL, :], bef_all[:, :], eye_sb[:, :])
                    bT_sb = pb2.tile([BL, 128], f32, tag="bT")
                    nc.vector.tensor_copy(bT_sb[:, :], psq[:BL, :])
                    bfl = pb2.tile([1, BL * 128], f32, tag="bfl")
                    nc.gpsimd.dma_start(bfl[:, :], bT_sb[:, :])
                    for half in range(2):
                        hs = slice(half * 512, (half + 1) * 512)
                        psr = psb.tile([128, 512], f32, tag="mmbig")
                        nc.tensor.matmul(psr[:, :], ones_sb[:, :], bfl[:, hs],
                                         start=True, stop=True)
                        nc.vector.tensor_copy(
                            brep[:, half * 4:(half + 1) * 4, :].rearrange(
                                "p b t -> p (b t)"), psr[:, :])

            # ---------- C: c_t + blend -> actT ----------
            with tc.tile_pool(name="pc2", bufs=1) as pp2, \
                 tc.tile_pool(name="phC", bufs=3) as pc:
                aT = pp2.tile([128, KC, ROWS], bf, tag="aT")
                an = pp2.tile([64, BL, H], bf, tag="an")
                nc.scalar.dma_start(an[:, :, :],
                                    an_d[:].rearrange("(b r) k -> r b k", r=64))
                for bg in range(2):
                    for hc in range(KC):
                        gs = slice(bg * 512, (bg + 1) * 512)
                        ps = psb.tile([128, 512], f32, tag="mmbig")
                        for j in range(4):
                            b = bg * 4 + j
                            nc.tensor.matmul(
                                ps[:, j * 128:(j + 1) * 128],
                                an[:49, b, hc * 128:(hc + 1) * 128],
                                alT[:49, b, :], start=True, stop=True)
                        # c_hat = c_t + beta*(s - c_t);  act = c_hat + hiddens
                        ctb = pc.tile([128, 512], bf, tag="ctb")
                        nc.vector.tensor_copy(ctb[:, :], ps[:, :])
                        tmp = pc.tile([128, 512], bf, tag="tmp")
                        nc.vector.tensor_sub(tmp[:, :], sT[:, hc, gs], ctb[:, :])
                        br = brep[:, bg * 4:(bg + 1) * 4, :].rearrange(
                            "p b t -> p (b t)")
                        nc.vector.tensor_mul(tmp[:, :], tmp[:, :], br)
                        nc.vector.tensor_add(tmp[:, :], tmp[:, :], ctb[:, :])
                        nc.vector.tensor_add(aT[:, hc, gs], tmp[:, :], hT[:, hc, gs])

                # ---------- D: scores = act @ Wm (+ bm) ----------
                with tc.tile_pool(name="phD", bufs=3) as pd, \
                     tc.tile_pool(name="phDo", bufs=2) as pdo:
                    for vc in range(NVC):
                        nv = min(NV, V1 - vc * NV)
                        wm_t = pd.tile([128, KC, NV], bf, tag="wmt")
                        nc.sync.dma_start(wm_t[:, :, :], wmp[vc])
                        if with_bias:
                            bmr_t = pd.tile([128, NV], f32, tag="bmrt")
                            nc.sync.dma_start(bmr_t[:, :nv], bmr_d[vc, :, :nv])
                        for rg in range(2):
                            ot = pdo.tile([128, 4, NV], f32, tag="ot")
                            for j in range(4):
                                rt = rg * 4 + j
                                ps = psb.tile([128, 512], f32, tag="mmbig")
                                for kc in range(KC):
                                    nc.tensor.matmul(
                                        ps[:, :nv],
                                        aT[:, kc, rt * 128:(rt + 1) * 128],
                                        wm_t[:, kc, :nv],
                                        start=(kc == 0), stop=(kc == KC - 1))
                                if with_bias:
                                    nc.vector.tensor_add(ot[:, j, :nv], ps[:, :nv],
                                                         bmr_t[:, :nv])
                                else:
                                    nc.vector.tensor_copy(ot[:, j, :nv], ps[:, :nv])
                            dst = sc_o[rg * 512:(rg + 1) * 512,
                                       vc * NV:vc * NV + nv].rearrange(
                                "(rb p) c -> p rb c", p=128)
                            eng = nc.scalar if (vc + rg) % 2 else nc.sync
                            eng.dma_start(dst, ot[:, :, :nv])

    nc.compile()
    return nc


def prep_core_inputs(x, hiddens, cells, att_feats, Wx, Wh, Wv, Wg, Ws, wh, Wm, bm):
    """Host-side prep: bf16 casts, transposes, pads, weight retiling."""
    def bt(a):  # [B,T,H] f32 -> bf16 [B, H, T]
        return np.ascontiguousarray(
            a.astype(BF16).reshape(B, T, H).transpose(0, 2, 1))

    xTt = bt(x)
    hTt = bt(hiddens)
    cTt = bt(cells)

    ap = np.zeros((B, 64, H), BF16)
    ap[:, :A, :] = att_feats.astype(BF16)
    atTt = np.ascontiguousarray(ap.transpose(0, 2, 1))  # [B, H, 64]

    def wtile(W):  # [H, H] -> [hc, p, kc, c]
        return np.ascontiguousarray(
            W.astype(BF16).reshape(KC, 128, KC, 128).transpose(2, 1, 0, 3))

    wm_pad = np.zeros((H, NVC * NV), BF16)
    wm_pad[:, :V1] = Wm.astype(BF16)
    wmp = np.ascontiguousarray(
        wm_pad.reshape(KC, 128, NVC, NV).transpose(2, 1, 0, 3))

    with_bias = bool(np.any(bm))
    shared = {
        "wxp": wtile(Wx), "whp": wtile(Wh),
        "wv": Wv.astype(BF16), "wg": Wg.astype(BF16), "ws": Ws.astype(BF16),
        "whr": np.ascontiguousarray(
            np.broadcast_to(wh.astype(np.float32)[None, :], (128, A))),
        "eye": np.eye(128, dtype=np.float32),
        "wmp": wmp,
    }
    if with_bias:
        bm_pad = np.zeros((NVC * NV,), np.float32)
        bm_pad[:V1] = bm.astype(np.float32)
        shared["bmr"] = np.ascontiguousarray(
            np.broadcast_to(bm_pad.reshape(NVC, 1, NV), (NVC, 128, NV)))

    def tocore(a, i):  # [B, H, N] -> [H, BL*N] for core i
        return np.ascontiguousarray(
            a[i * BL:(i + 1) * BL].transpose(1, 0, 2).reshape(H, -1))

    def core_map(i):
        m = dict(shared)
        m["xT"] = tocore(xTt, i)
        m["hT"] = tocore(hTt, i)
        m["cT"] = tocore(cTt, i)
        m["atT"] = tocore(atTt, i)
        m["an"] = ap[i * BL:(i + 1) * BL].reshape(BL * 64, H)
        return m

    return core_map, with_bias


def kernel(x, hiddens, cells, att_feats, Wx, Wh, Wv, Wg, Ws, wh, Wm, bm):
    from concourse.bass_utils import run_bass_kernel_spmd

    core_map, with_bias = prep_core_inputs(x, hiddens, cells, att_feats, Wx, Wh,
                                           Wv, Wg, Ws, wh, Wm, bm)
    nc = build_nc(with_bias=with_bias)
    in_maps = [core_map(i) for i in range(N_CORES)]
    res = run_bass_kernel_spmd(nc, in_maps, core_ids=list(range(N_CORES)))

    scores = np.concatenate(
        [res.results[i]["scores"].reshape(BL, T, V1) for i in range(N_CORES)], axis=0)
    alpha = np.concatenate(
        [res.results[i]["alpha"] for i in range(N_CORES)], axis=0)
    beta = np.concatenate(
        [res.results[i]["beta"] for i in range(N_CORES)], axis=0)
    return scores, alpha, beta
